# revision 13
# baseline (speedup 1.0000x reference)
"""Self-contained Trainium2 Bass kernel for the 3-layer stacked GRU encoder
(nn_NoisyGRUSeq2SeqWithFeatures).

Strategy: 8-way MODEL-parallel (output-channel sharding) so all 105MB of
weights stay SBUF-resident across the 64-step recurrence; the full batch
(B=128) is replicated on every core so every matmul runs with a full
128-wide stationary operand.  Per wave we run a layer-skewed schedule
(L0 at t, L1 at t-1, L2 at t-2) which lets the six per-step AllGathers
fuse into two.  Variable sequence lengths are handled by adding +30 to
the pre-sigmoid gate logits of finished samples (z -> 1 freezes h).
"""

import numpy as np

SIZES = (512, 1024, 2048)
EMB = 32
VOC = 40
LATENT = 512
B, S = 128, 64
NCORES = 8

# per-core output slice sizes per layer
SL = tuple(sz // NCORES for sz in SIZES)  # (64, 128, 256)
HLOC = sum(SL)  # 448 columns of per-core h state
F32 = None  # set after mybir import
SKIP_COLL = False  # debug: skip collectives to isolate their cost


def _sbufize(w: np.ndarray, tp: int = 128) -> np.ndarray:
    """[nk*tp, ncols] row-tiled weight -> SBUF layout [tp, nk*ncols]."""
    nk = w.shape[0] // tp
    assert w.shape[0] == nk * tp
    return (
        np.ascontiguousarray(w.reshape(nk, tp, w.shape[1]).transpose(1, 0, 2))
        .reshape(tp, nk * w.shape[1])
        .astype(np.float32)
    )


def prepack(inputs: dict) -> list[dict]:
    """Build per-core in_maps from the full (unsharded) problem inputs."""
    seqs = np.asarray(inputs["input_seqs"])
    lens = np.asarray(inputs["input_lens"])
    emb = np.asarray(inputs["emb"], np.float32)
    Kg = [np.asarray(inputs[f"Kg{l}"], np.float32) for l in range(3)]
    bg = [np.asarray(inputs[f"bg{l}"], np.float32) for l in range(3)]
    Kc = [np.asarray(inputs[f"Kc{l}"], np.float32) for l in range(3)]
    bc = [np.asarray(inputs[f"bc{l}"], np.float32) for l in range(3)]
    W_enc = np.asarray(inputs["W_enc"], np.float32)
    b_enc = np.asarray(inputs["b_enc"], np.float32)

    # shared tensors
    x_emb = emb[seqs]  # [B,S,EMB]
    xembT = np.zeros((EMB + 1, S * B), np.float32)
    for t in range(S):
        xembT[:EMB, t * B : (t + 1) * B] = x_emb[:, t, :].T
    xembT[EMB, :] = 1.0
    maskb = np.zeros((B, S), np.float32)
    for t in range(S):
        maskb[:, t] = np.where(t < lens, 0.0, 30.0)
    ident = np.eye(128, dtype=np.float32)
    ones_row = np.ones((1, B), np.float32)

    xin = (EMB, SIZES[0], SIZES[1])  # x-input width per layer
    in_maps = []
    for i in range(NCORES):
        m = {
            "xembT": xembT,
            "maskb": maskb,
            "ident": ident,
            "ones_row": ones_row,
        }
        ccols = {}
        for l in range(3):
            sl, cout = SL[l], SIZES[l]
            rcols = sl * i + np.arange(sl)
            gcols = np.concatenate([rcols, cout + rcols])
            ccols[l] = rcols
            cin = xin[l]
            if l == 0:
                m["kg0x"] = np.vstack([Kg[0][:cin, gcols], bg[0][gcols][None, :]])
                m["kc0x"] = np.vstack([Kc[0][:cin, ccols[0]], bc[0][ccols[0]][None, :]])
            else:
                tpx = 64 if l == 1 else 128  # L1 x-input is h0 (64-row rank chunks)
                m[f"kg{l}x"] = _sbufize(Kg[l][:cin, gcols], tpx)
                m[f"kc{l}x"] = _sbufize(Kc[l][:cin, ccols[l]], tpx)
                m[f"bg{l}row"] = bg[l][gcols][None, :].astype(np.float32)
                m[f"bc{l}row"] = bc[l][ccols[l]][None, :].astype(np.float32)
            tp = 64 if l == 0 else 128
            m[f"kg{l}h"] = _sbufize(Kg[l][cin:, gcols], tp)
            m[f"kc{l}h"] = _sbufize(Kc[l][cin:, ccols[l]], tp)
        # W_enc rows for this core's h slices, in h_loc order, padded to 512
        wrows = np.concatenate(
            [
                512 + 128 * i + np.arange(128),
                1536 + 256 * i + np.arange(256),
                64 * i + np.arange(64),
            ]
        )
        wenc = np.zeros((512, LATENT), np.float32)
        wenc[:HLOC] = W_enc[wrows]
        m["wenc"] = _sbufize(wenc)
        m["benc_row"] = (b_enc / NCORES)[None, :].astype(np.float32)
        in_maps.append(m)
    return in_maps


def build(n_waves=S + 2):
    import concourse.bass as bass
    import concourse.bacc as bacc
    import concourse.tile as tile
    import concourse.mybir as mybir

    f32 = mybir.dt.float32
    AF = mybir.ActivationFunctionType
    nc = bacc.Bacc("TRN2", target_bir_lowering=False, debug=False, num_devices=NCORES)

    # h_loc column layout: [h1 (0:128) | h2 (128:384) | h0 (384:448)]
    # rank chunk (transposed, padded to 512 rows): same order + 64 pad rows
    dshapes = {
        "xembT": [EMB + 1, S * B],
        "maskb": [B, S],
        "ident": [128, 128],
        "ones_row": [1, B],
        "kg0x": [EMB + 1, 2 * SL[0]],
        "kc0x": [EMB + 1, SL[0]],
        "kg0h": [64, 8 * 2 * SL[0]],
        "kc0h": [64, 8 * SL[0]],
        "kg1x": [64, 8 * 2 * SL[1]],
        "kc1x": [64, 8 * SL[1]],
        "kg1h": [128, 8 * 2 * SL[1]],
        "kc1h": [128, 8 * SL[1]],
        "bg1row": [1, 2 * SL[1]],
        "bc1row": [1, SL[1]],
        "kg2x": [128, 8 * 2 * SL[2]],
        "kc2x": [128, 8 * SL[2]],
        "kg2h": [128, 16 * 2 * SL[2]],
        "kc2h": [128, 16 * SL[2]],
        "bg2row": [1, 2 * SL[2]],
        "bc2row": [1, SL[2]],
        "wenc": [128, 4 * LATENT],
        "benc_row": [1, LATENT],
    }
    dram = {k: nc.dram_tensor(k, v, f32, kind="ExternalInput") for k, v in dshapes.items()}
    out_d = nc.dram_tensor("out", [B, LATENT], f32, kind="ExternalOutput")

    with tile.TileContext(nc) as tc:
        with (
            tc.tile_pool(name="wts", bufs=1) as wp,
            tc.tile_pool(name="acts", bufs=1) as ap,
            tc.tile_pool(name="hbuf", bufs=1) as hp,
            tc.tile_pool(name="stg", bufs=2) as sp,
            tc.tile_pool(name="gates", bufs=1) as gp,
            tc.tile_pool(name="psg", bufs=1, space="PSUM") as psg,
            tc.tile_pool(name="psc", bufs=1, space="PSUM") as psc,
            tc.tile_pool(name="pst", bufs=2, space="PSUM") as pst,
            tc.tile_pool(name="dram", bufs=2, space="DRAM") as dp,
        ):
            w = {}
            for k in dshapes:
                t = wp.tile(dshapes[k], f32, name=f"w_{k}")
                nc.sync.dma_start(t[:], dram[k][:])
                w[k] = t

            def wt(name, ncols, j, tp=128):
                return w[name][0:tp, j * ncols : (j + 1) * ncols]

            h_loc = ap.tile([B, HLOC], f32, name="h_loc")
            rh_loc = ap.tile([B, HLOC], f32, name="rh_loc")
            nc.vector.memset(h_loc[:], 0.0)
            nc.vector.memset(rh_loc[:], 0.0)

            def fresh_hT(tag):
                return hp.tile([128, NCORES * 4 * B], f32, name=f"{tag}T", tag=f"{tag}T")

            def RK(t, r, k):
                return t[:, (4 * r + k) * B : (4 * r + k + 1) * B]

            def RK64(t, r):
                return t[0:64, (4 * r + 3) * B : (4 * r + 3) * B + B]

            hT = fresh_hT("h")
            nc.vector.memset(hT[:], 0.0)

            def stage_and_gather(src_loc, tag, wv):
                pt = pst.tile([128, 512], f32, name=f"pt_{tag}{wv}", tag="pt")
                for k in range(4):
                    csz = 128 if k < 3 else HLOC - 384
                    nc.tensor.transpose(
                        pt[:csz, k * 128 : k * 128 + 128],
                        src_loc[:, k * 128 : k * 128 + csz],
                        w["ident"][:],
                    )
                stg = sp.tile([128, 512], f32, name=f"stg_{tag}{wv}", tag=f"stg{tag}")
                nc.vector.tensor_copy(stg[:], pt[:])
                agin = dp.tile([128, 512], f32, name=f"agin_{tag}{wv}", tag=f"agin{tag}")
                agout = dp.tile(
                    [NCORES * 128, 512], f32,
                    name=f"agout_{tag}{wv}", tag=f"agout{tag}", addr_space="Shared",
                )
                nc.sync.dma_start(agin[:], stg[:])
                nc.gpsimd.collective_compute(
                    "AllGather",
                    mybir.AluOpType.bypass,
                    replica_groups=[list(range(NCORES))],
                    ins=[agin[:]],
                    outs=[agout[:]],
                )
                gT = fresh_hT(tag)
                for r in range(NCORES):
                    nc.sync.dma_start(
                        gT[:, r * 512 : (r + 1) * 512],
                        agout[r * 128 : (r + 1) * 128, :],
                    )
                return gT

            for wv in range(n_waves):
                t0, t1, t2 = wv, wv - 1, wv - 2

                # ---------------- gates ----------------
                pg2 = psg.tile([B, 2 * SL[2]], f32, name=f"pg2_{wv}", tag="pg2")
                pg01 = psg.tile([B, 2 * (SL[0] + SL[1])], f32, name=f"pg01_{wv}", tag="pg01")
                if 0 <= t2 < S:
                    nc.tensor.matmul(pg2[:], w["ones_row"][:], w["bg2row"][:], start=True, stop=False)
                    for r in range(NCORES):
                        nc.tensor.matmul(pg2[:], RK(hT, r, 0), wt("kg2x", 2 * SL[2], r),
                                         start=False, stop=False)
                    for r in range(NCORES):
                        for a in range(2):
                            nc.tensor.matmul(pg2[:], RK(hT, r, 1 + a), wt("kg2h", 2 * SL[2], 2 * r + a),
                                             start=False, stop=(r == NCORES - 1 and a == 1))
                if 0 <= t1 < S:
                    nc.tensor.matmul(pg01[:, 128:384], w["ones_row"][:], w["bg1row"][:], start=True, stop=False)
                    for r in range(NCORES):
                        nc.tensor.matmul(pg01[:, 128:384], RK64(hT, r), wt("kg1x", 2 * SL[1], r, 64),
                                         start=False, stop=False)
                    for r in range(NCORES):
                        nc.tensor.matmul(pg01[:, 128:384], RK(hT, r, 0), wt("kg1h", 2 * SL[1], r),
                                         start=False, stop=(r == NCORES - 1))
                if t0 < S:
                    nc.tensor.matmul(pg01[:, 0:128], w["xembT"][:, t0 * B : (t0 + 1) * B],
                                     w["kg0x"][:], start=True, stop=False)
                    for r in range(NCORES):
                        nc.tensor.matmul(pg01[:, 0:128], RK64(hT, r), wt("kg0h", 2 * SL[0], r, 64),
                                         start=False, stop=(r == NCORES - 1))

                # ---------------- sigmoid + r*h ----------------
                g2sb = gp.tile([B, 2 * SL[2]], f32, name=f"g2sb_{wv}", tag="g2sb")
                g01sb = gp.tile([B, 2 * (SL[0] + SL[1])], f32, name=f"g01sb_{wv}", tag="g01sb")
                if 0 <= t2 < S:
                    nc.scalar.activation(g2sb[:], pg2[:], AF.Sigmoid, bias=w["maskb"][:, t2 : t2 + 1])
                    nc.vector.tensor_mul(rh_loc[:, 128:384], g2sb[:, 0 : SL[2]], h_loc[:, 128:384])
                if 0 <= t1 < S:
                    nc.scalar.activation(g01sb[:, 128:384], pg01[:, 128:384], AF.Sigmoid,
                                         bias=w["maskb"][:, t1 : t1 + 1])
                    nc.vector.tensor_mul(rh_loc[:, 0:128], g01sb[:, 128 : 128 + SL[1]], h_loc[:, 0:128])
                if t0 < S:
                    nc.scalar.activation(g01sb[:, 0:128], pg01[:, 0:128], AF.Sigmoid,
                                         bias=w["maskb"][:, t0 : t0 + 1])
                    nc.vector.tensor_mul(rh_loc[:, 384:HLOC], g01sb[:, 0 : SL[0]], h_loc[:, 384:HLOC])

                # ---------------- AG(rh) ----------------
                rhT = stage_and_gather(rh_loc, "r", wv)

                # ---------------- candidates ----------------
                pc = psc.tile([B, HLOC], f32, name=f"pc_{wv}", tag="pc")
                if 0 <= t1 < S:
                    nc.tensor.matmul(pc[:, 0:128], w["ones_row"][:], w["bc1row"][:], start=True, stop=False)
                    for r in range(NCORES):
                        nc.tensor.matmul(pc[:, 0:128], RK64(hT, r), wt("kc1x", SL[1], r, 64),
                                         start=False, stop=False)
                    for r in range(NCORES):
                        nc.tensor.matmul(pc[:, 0:128], RK(rhT, r, 0), wt("kc1h", SL[1], r),
                                         start=False, stop=(r == NCORES - 1))
                if 0 <= t2 < S:
                    nc.tensor.matmul(pc[:, 128:384], w["ones_row"][:], w["bc2row"][:], start=True, stop=False)
                    for r in range(NCORES):
                        nc.tensor.matmul(pc[:, 128:384], RK(hT, r, 0), wt("kc2x", SL[2], r),
                                         start=False, stop=False)
                    for r in range(NCORES):
                        for a in range(2):
                            nc.tensor.matmul(pc[:, 128:384], RK(rhT, r, 1 + a), wt("kc2h", SL[2], 2 * r + a),
                                             start=False, stop=(r == NCORES - 1 and a == 1))
                if t0 < S:
                    nc.tensor.matmul(pc[:, 384:HLOC], w["xembT"][:, t0 * B : (t0 + 1) * B],
                                     w["kc0x"][:], start=True, stop=False)
                    for r in range(NCORES):
                        nc.tensor.matmul(pc[:, 384:HLOC], RK64(rhT, r), wt("kc0h", SL[0], r, 64),
                                         start=False, stop=(r == NCORES - 1))

                # ---------------- tanh + h_new ----------------
                csb = gp.tile([B, HLOC], f32, name=f"csb_{wv}", tag="csb")
                nc.scalar.activation(csb[:], pc[:], AF.Tanh)
                tmp2 = gp.tile([B, SL[2]], f32, name=f"tmp2_{wv}", tag="tmp2")
                zsl = {
                    0: (g01sb, SL[0], 384, HLOC),
                    1: (g01sb, 256, 0, 128),
                    2: (g2sb, SL[2], 128, 384),
                }
                for l, tl in ((0, t0), (1, t1), (2, t2)):
                    if tl < 0 or tl >= S:
                        continue
                    gt, zoff, a, b2 = zsl[l]
                    sw = b2 - a
                    nc.vector.tensor_sub(tmp2[:, :sw], h_loc[:, a:b2], csb[:, a:b2])
                    nc.vector.tensor_mul(tmp2[:, :sw], gt[:, zoff : zoff + sw], tmp2[:, :sw])
                    nc.vector.tensor_add(h_loc[:, a:b2], tmp2[:, :sw], csb[:, a:b2])

                # ---------------- AG(h) ----------------
                hT = stage_and_gather(h_loc, "h", wv)

            # ---------------- final projection ----------------
            ptf = pst.tile([128, 512], f32, name="ptf", tag="pt")
            for k in range(4):
                csz = 128 if k < 3 else HLOC - 384
                nc.tensor.transpose(
                    ptf[:csz, k * 128 : k * 128 + 128],
                    h_loc[:, k * 128 : k * 128 + csz],
                    w["ident"][:],
                )
            hsf = sp.tile([128, 512], f32, name="hsf", tag="stgh")
            nc.vector.tensor_copy(hsf[:], ptf[:])
            nc.vector.memset(hsf[64:128, 384:512], 0.0)
            pz = psg.tile([B, LATENT], f32, name="pz", tag="pg2")
            nc.tensor.matmul(pz[:], w["ones_row"][:], w["benc_row"][:], start=True, stop=False)
            for k in range(4):
                nc.tensor.matmul(pz[:], hsf[:, k * 128 : (k + 1) * 128],
                                 wt("wenc", LATENT, k), start=False, stop=(k == 3))
            zsb = gp.tile([B, LATENT], f32, name="zsb", tag="g2sb")
            nc.vector.tensor_copy(zsb[:], pz[:])
            arin = dp.tile([B, LATENT], f32, name="arin")
            arout = dp.tile([B, LATENT], f32, name="arout", addr_space="Shared")
            nc.sync.dma_start(arin[:], zsb[:])
            nc.gpsimd.collective_compute(
                "AllReduce",
                mybir.AluOpType.add,
                replica_groups=[list(range(NCORES))],
                ins=[arin[:]],
                outs=[arout[:]],
            )
            zfull = gp.tile([B, LATENT], f32, name="zfull", tag="csb")
            nc.sync.dma_start(zfull[:], arout[:])
            ofin = gp.tile([B, LATENT], f32, name="ofin", tag="g01sb")
            nc.scalar.activation(ofin[:], zfull[:], AF.Tanh)
            nc.sync.dma_start(out_d[:], ofin[:])

    nc.compile()
    return nc


_NC_CACHE = {}


def kernel(**inputs) -> np.ndarray:
    from concourse import bass_utils

    if "nc" not in _NC_CACHE:
        _NC_CACHE["nc"] = build()
    nc = _NC_CACHE["nc"]
    in_maps = prepack(inputs)
    res = bass_utils.run_bass_kernel_spmd(nc, in_maps, core_ids=list(range(NCORES)))
    return np.asarray(res.results[0]["out"], np.float32)
